# revision 30
# baseline (speedup 1.0000x reference)
"""Bahdanau additive attention on 8 Trainium2 cores — odd-harmonic kernel.

reference:
    proj_dec = dec @ Ws + bs            [B, DEC, A]
    proj_enc = enc @ Wh                 [B, ENC, A]
    logits[b,d,e] = sum_a v[a] * tanh(proj_dec[b,d,a] + proj_enc[b,e,a])
    attn = renormalized softmax(logits, axis=e) * mask
    ctx = attn @ enc                    [B, DEC, H]
    returns (ctx, attn)

Sharding: 8 cores = (batch b in 0..3) x (decoder half in 0..1); each core does
128 decoder rows against the full encoder of its batch. Fully sync-free: no
cross-core traffic (collectives under this runtime pay a launch-skew barrier
that dwarfs their payload).

Math: tanh(z) ~= sum_{k in {1,3,5,7,9}} b_k sin(k om z) — odd harmonics only
(tanh is a smoothed square wave, so even harmonics contribute ~nothing; L and
b_k are a least-squares fit on [-6.19, 6.19], max fit err 4.9e-3 — half the
error of 8 consecutive harmonics at 5/8 the cost). Angle addition makes the
score one bf16 matmul with contraction A*5*2. Harmonics come from the
stride-2 Chebyshev recurrence with t2 = 2cos(2u):
    s3 = (t2+1) s1,  c3 = (t2-1) c1,  x_k = t2 * x_{k-2} - x_{k-4}
seeded by ACT Sin; t2 and t2+-1 derive from sq = s1^2 via fused tensor_scalar
ops (4x DVE mode). sin/cos chains live interleaved in one tile so each
recurrence step is one elementwise op over both.

Engine split: the e-side chains saturate DVE (the critical resource); the
entire d-side chain pipeline runs on the otherwise-idle GpSimd engine in
parallel. e-side seeds read proj_enc straight out of PSUM (no evacuation
copy). Softmax skips the max-subtraction (|logits| <= sum|v_a|*1.05 ~= 4.3,
f32-exp-safe); Exp emits bf16 + fused row-sum, and 1/sum is folded into the
ctx PSUM evacuation. Inputs are staged host-side as bf16 and pre-transposed
(encT, decT), so no PE transposes feed the projections.
"""

import numpy as np

import concourse.bass as bass
import concourse.mybir as mybir
import concourse.tile as tile
from concourse import bacc
from concourse.bass_utils import run_bass_kernel_spmd
from concourse.masks import make_identity

B, ENC, DEC, H, A = 4, 1024, 256, 1024, 256
P = 128
HK = H // P    # 8 contraction tiles over hidden dim
AT = A // P    # 2 tiles over attention dim
EK = ENC // P  # 8 encoder tiles
DH = 128       # decoder rows per core
NB = 512       # psum bank free-dim (f32)
F32 = mybir.dt.float32
BF16 = mybir.dt.bfloat16
AF = mybir.ActivationFunctionType
ALU = mybir.AluOpType

KS = (1, 3, 5, 7, 9)
NK = len(KS)
OMEGA = float(np.pi / 8.95)

_CACHE = {}


def _fit_coeffs():
    z = np.linspace(-6.19, 6.19, 20001)
    mat = np.sin(np.outer(z, np.array(KS) * OMEGA))
    b = np.linalg.lstsq(mat, np.tanh(z), rcond=None)[0]
    return [float(x) for x in b]


def _build_kernel(mask_ones: bool):
    bco = _fit_coeffs()
    nc = bacc.Bacc("TRN2", target_bir_lowering=False, debug=False)
    # all big inputs are host-tiled to [P, ...contiguous] so each DMA is 128
    # large descriptors instead of ~1024 small ones
    encT = nc.dram_tensor("encT", [P, 2, HK, NB], BF16, kind="ExternalInput").ap()
    encf = nc.dram_tensor("encf", [P, EK, H], BF16, kind="ExternalInput").ap()
    decT = nc.dram_tensor("decT", [P, HK, DH], BF16, kind="ExternalInput").ap()
    wh = nc.dram_tensor("wh", [P, HK, A], BF16, kind="ExternalInput").ap()
    ws = nc.dram_tensor("ws", [P, HK, A], BF16, kind="ExternalInput").ap()
    bsv = nc.dram_tensor("bsv", [P, 2 * AT], F32, kind="ExternalInput").ap()
    maskl = nc.dram_tensor("maskl", [1, ENC], F32, kind="ExternalInput").ap()
    ctx_out = nc.dram_tensor("ctx_out", [DH, H], BF16, kind="ExternalOutput").ap()
    attn_out = nc.dram_tensor("attn_out", [DH, ENC], BF16, kind="ExternalOutput").ap()

    def bcast(t, n):
        return bass.AP(tensor=t.tensor, offset=t.offset, ap=[[0, P], [1, n]])

    with tile.TileContext(nc) as tc:
        with (
            tc.tile_pool(name="big", bufs=1) as big,
            tc.tile_pool(name="small", bufs=1) as small,
            tc.tile_pool(name="ech", bufs=5) as ech,
            tc.tile_pool(name="ps_mm", bufs=3, space="PSUM") as ps_mm,
            tc.tile_pool(name="ps_lg", bufs=1, space="PSUM") as ps_lg,
            tc.tile_pool(name="ps_w", bufs=1, space="PSUM") as ps_w,
            tc.tile_pool(name="ps_tr", bufs=1, space="PSUM") as ps_tr,
            tc.tile_pool(name="ps_cx", bufs=1, space="PSUM") as ps_cx,
        ):
            # ---- loads. Small d-side tensors first (scalar queue) so pd can
            # start immediately; encT halves next (sync); encf is dispatched
            # much later in program order — ctx needs it only near the end,
            # and an early dispatch would starve everything else on the bus.
            decT_sb = big.tile([P, HK, DH], BF16)
            nc.scalar.dma_start(out=decT_sb, in_=decT)
            ws_sb = big.tile([P, HK, A], BF16)
            nc.scalar.dma_start(out=ws_sb, in_=ws)
            wh_sb = big.tile([P, HK, A], BF16)
            nc.scalar.dma_start(out=wh_sb, in_=wh)
            bsv_sb = small.tile([P, 2 * AT], F32)
            nc.scalar.dma_start(out=bsv_sb, in_=bsv)
            bs_sb = bsv_sb[:, 0:AT]
            v_sb = bsv_sb[:, AT:2 * AT]
            encT_sb = big.tile([P, 2, HK, NB], BF16)
            nc.sync.dma_start(out=encT_sb[:, 0], in_=encT[:, 0])
            nc.sync.dma_start(out=encT_sb[:, 1], in_=encT[:, 1])
            if not mask_ones:
                mask_sb = big.tile([P, ENC], F32)
                nc.sync.dma_start(out=mask_sb, in_=bcast(maskl, ENC))

            consts = small.tile([P, 2], F32)
            nc.vector.memset(consts[:, 0:1], OMEGA)
            nc.vector.memset(consts[:, 1:2], float(np.pi / 2))
            om_ap = consts[:, 0:1]
            halfpi_ap = consts[:, 1:2]

            fd = big.tile([P, AT, NK, 2, DH], BF16)

            def pe_warm(n):
                for _ in range(n):
                    pw = ps_w.tile([P, NB], F32, tag="warm")
                    nc.tensor.matmul(
                        pw[:, :2 * DH], fd[:, 0, 0, 0], fd[:, 0, 0], start=True,
                        stop=True, skip_group_check=True,
                    )

            pe_warm(4)

            def chain_steps(eng, sc1, n, pool, tag, slices, mm_hook=None):
                """Odd-harmonic sin/cos arrays from seeded sc1 (see module doc).

                Returns {k: [P, AT, 2, n] bf16}; recurrence math runs on `eng`,
                emitted independently per column slice so each slice's chain
                (and its logits matmuls) can start as soon as that slice's
                seeds exist.
                """
                sq = big.tile([P, AT, n], BF16, tag=tag + "sq")
                t2 = big.tile([P, AT, 2, n], BF16, tag=tag + "t2")
                t2pm = big.tile([P, AT, 2, n], BF16, tag=tag + "t2pm")
                sc3 = pool.tile([P, AT, 2, n], BF16, tag=tag)
                sc = {1: sc1, 3: sc3}
                for k in (5, 7, 9):
                    sc[k] = pool.tile([P, AT, 2, n], BF16, tag=tag, name=f"{tag}sc{k}")
                # steps interleave the (independent) column slices so the
                # in-order engine fills one slice's recurrence latency with
                # the other slice's work. DVE tensor_scalar runs 1x on HW, so
                # the t2 builders go to ACT as Copy(scale,bias) for the big
                # e-side; gpsimd keeps its own (immediate scalars are fine
                # there).
                for si, sl in enumerate(slices):
                    eng.tensor_mul(sq[:, :, sl], sc1[:, :, 0, sl], sc1[:, :, 0, sl])
                    if mm_hook:
                        mm_hook(0, sc1, si)
                for si, sl in enumerate(slices):
                    for ph in range(2):
                        eng.tensor_scalar(
                            out=t2pm[:, :, ph, sl], in0=sq[:, :, sl], scalar1=-4.0,
                            scalar2=(3.0 if ph == 0 else 1.0), op0=ALU.mult, op1=ALU.add,
                        )
                    for ph in range(2):
                        eng.tensor_scalar(
                            out=t2[:, :, ph, sl], in0=sq[:, :, sl], scalar1=-4.0,
                            scalar2=2.0, op0=ALU.mult, op1=ALU.add,
                        )
                for si, sl in enumerate(slices):
                    eng.tensor_mul(sc3[:, :, :, sl], t2pm[:, :, :, sl], sc1[:, :, :, sl])
                    if mm_hook:
                        mm_hook(1, sc3, si)
                for ki, k in enumerate((5, 7, 9)):
                    t = sc[k]
                    for si, sl in enumerate(slices):
                        eng.tensor_mul(t[:, :, :, sl], t2[:, :, :, sl], sc[k - 2][:, :, :, sl])
                    for si, sl in enumerate(slices):
                        eng.tensor_sub(t[:, :, :, sl], t[:, :, :, sl], sc[k - 4][:, :, :, sl])
                        if mm_hook:
                            mm_hook(2 + ki, t, si)
                return sc

            # ---- d-side: proj_dec^T [a,(at,d)]; chains on GpSimd so DVE
            # stays free for the (4x bigger) e-side chains.
            pd_sb = big.tile([P, AT, DH], F32)
            for at in range(AT):
                pp = ps_mm.tile([P, NB], F32, tag="mm")
                for hk in range(HK):
                    nc.tensor.matmul(
                        pp[:, :DH],
                        ws_sb[:, hk, at * P:(at + 1) * P],
                        decT_sb[:, hk, :],
                        start=(hk == 0),
                        stop=(hk == HK - 1),
                    )
                nc.vector.tensor_scalar_add(pd_sb[:, at], pp[:, :DH], bs_sb[:, at:at + 1])

            pe_warm(6)
            dsc1 = ech.tile([P, AT, 2, DH], BF16, tag="d")
            nc.scalar.activation(out=dsc1[:, :, 0], in_=pd_sb, func=AF.Sin, scale=om_ap)
            nc.scalar.activation(out=dsc1[:, :, 1], in_=pd_sb, func=AF.Sin, scale=om_ap, bias=halfpi_ap)
            vb = small.tile([P, AT, NK], F32)
            for ki in range(NK):
                for at in range(AT):
                    nc.vector.tensor_scalar_mul(
                        vb[:, at, ki:ki + 1], v_sb[:, at:at + 1], bco[ki]
                    )
            dsc = chain_steps(nc.gpsimd, dsc1, DH, ech, "d", [slice(0, DH)])
            for ki in range(NK):
                for at in range(AT):
                    nc.vector.tensor_scalar_mul(
                        fd[:, at, ki], dsc[KS[ki]][:, at], vb[:, at, ki:ki + 1]
                    )

            # ---- e-side: proj_enc^T per PSUM quadrant, seeds read PSUM ----
            encf_sb = big.tile([P, EK, H], BF16)
            nc.sync.dma_start(out=encf_sb, in_=encf)
            esc1 = ech.tile([P, AT, 2, ENC], BF16, tag="e")
            for he in range(2):
                for at in range(AT):
                    pp = ps_mm.tile([P, NB], F32, tag="mm")
                    for hk in range(HK):
                        nc.tensor.matmul(
                            pp,
                            wh_sb[:, hk, at * P:(at + 1) * P],
                            encT_sb[:, he, hk, :],
                            start=(hk == 0),
                            stop=(hk == HK - 1),
                        )
                    sl = slice(he * NB, (he + 1) * NB)
                    nc.scalar.activation(out=esc1[:, at, 0, sl], in_=pp, func=AF.Sin, scale=om_ap)
                    nc.scalar.activation(out=esc1[:, at, 1, sl], in_=pp, func=AF.Sin, scale=om_ap, bias=halfpi_ap)

            # ---- e-side chains with logits matmuls interleaved ----
            lg_psum = ps_lg.tile([P, 2, NB], F32)

            def logits_mm(ki, esc_k, he):
                for ph in range(2):
                    for at in range(AT):
                        nc.tensor.matmul(
                            lg_psum[:, he],
                            fd[:, at, ki, ph],
                            esc_k[:, at, 1 - ph, he * NB:(he + 1) * NB],
                            start=(ki == 0 and ph == 0 and at == 0),
                            stop=(ki == NK - 1 and ph == 1 and at == AT - 1),
                            skip_group_check=True,
                        )

            chain_steps(
                nc.vector, esc1, ENC, ech, "e",
                [slice(0, NB), slice(NB, ENC)], mm_hook=logits_mm,
            )

            # ---- softmax, without max-subtraction: |logits| <= sum|v_a|*1.05
            # (~4.3 here), so exp() is f32-safe unshifted. Exp writes bf16
            # directly and emits the row-sum in the same pass; 1/sum is folded
            # into the ctx PSUM evacuation and a parallel attn-output scale.
            pe_warm(8)
            expt_bf = big.tile([P, ENC], BF16)
            rowsum = small.tile([P, 1], F32)
            if mask_ones:
                nc.scalar.activation(
                    out=expt_bf, in_=lg_psum.rearrange("p h e -> p (h e)"),
                    func=AF.Exp, accum_out=rowsum,
                )
            else:
                nc.scalar.activation(
                    out=expt_bf, in_=lg_psum.rearrange("p h e -> p (h e)"), func=AF.Exp
                )
                nc.vector.tensor_mul(expt_bf, expt_bf, mask_sb)
                nc.vector.tensor_reduce(
                    out=rowsum, in_=expt_bf, axis=mybir.AxisListType.X, op=ALU.add
                )
            rinv = small.tile([P, 1], F32)
            nc.vector.reciprocal(rinv, rowsum)
            attn_bf = big.tile([P, ENC], BF16)
            nc.scalar.mul(attn_bf, expt_bf, rinv)
            nc.sync.dma_start(out=attn_out, in_=attn_bf)

            # ---- ctx = attn @ enc (unnormalized; rinv applied at evac) ----
            ident_f = small.tile([P, P], F32)
            make_identity(nc, ident_f)
            ident = small.tile([P, P], BF16)
            nc.vector.tensor_copy(ident, ident_f)
            attnT = big.tile([P, EK, P], BF16)
            for g in range(2):
                pt = ps_tr.tile([P, 4, P], BF16)
                for j in range(4):
                    ek = g * 4 + j
                    nc.tensor.transpose(pt[:, j], expt_bf[:, ek * P:(ek + 1) * P], ident)
                for j in range(4):
                    nc.vector.tensor_copy(attnT[:, g * 4 + j], pt[:, j])
            ctx_sb = big.tile([P, H], BF16)
            for nh in range(2):
                pc = ps_cx.tile([P, NB], F32, tag="cx")
                for ek in range(EK):
                    nc.tensor.matmul(
                        pc,
                        attnT[:, ek],
                        encf_sb[:, ek, nh * NB:(nh + 1) * NB],
                        start=(ek == 0),
                        stop=(ek == EK - 1),
                    )
                nc.scalar.mul(ctx_sb[:, nh * NB:(nh + 1) * NB], pc, rinv)
                nc.sync.dma_start(
                    out=bass.AP(
                        tensor=ctx_out.tensor, offset=ctx_out.offset + nh * NB,
                        ap=[[H, P], [1, NB]],
                    ),
                    in_=ctx_sb[:, nh * NB:(nh + 1) * NB],
                )

    nc.compile()
    return nc


def kernel(encoded_seq, decoder_state, input_pad_mask, Wh, Ws, bs, v, trace=False):
    import ml_dtypes

    bf16 = ml_dtypes.bfloat16
    encoded_seq = np.asarray(encoded_seq, dtype=np.float32)
    decoder_state = np.asarray(decoder_state, dtype=np.float32)
    input_pad_mask = np.asarray(input_pad_mask, dtype=np.float32)
    Wh_b = np.ascontiguousarray(np.asarray(Wh, np.float32).astype(bf16))
    Ws_b = np.ascontiguousarray(np.asarray(Ws, np.float32).astype(bf16))
    bs2 = np.asarray(bs, dtype=np.float32).reshape(AT, P)
    v2 = np.asarray(v, dtype=np.float32).reshape(AT, P)
    # host-tiled [P, (bs_at0, bs_at1, v_at0, v_at1)] — plain contiguous load
    bsv = np.ascontiguousarray(np.concatenate([bs2.T, v2.T], axis=1))

    mask_ones = bool(np.all(input_pad_mask == 1.0))
    key = ("nc", mask_ones)
    if key not in _CACHE:
        _CACHE[key] = _build_kernel(mask_ones)
    nc = _CACHE[key]

    def tile_rows(x, k):
        # [k*P, n] -> [P, k, n] per-partition-contiguous
        n = x.shape[1]
        return np.ascontiguousarray(x.reshape(k, P, n).transpose(1, 0, 2))

    in_maps = []
    enc_bf = [encoded_seq[b].astype(bf16) for b in range(B)]
    encf_t = [tile_rows(e, EK) for e in enc_bf]
    encT_t = []
    for e in enc_bf:
        et = tile_rows(np.ascontiguousarray(e.T), HK)       # [P, HK, ENC]
        encT_t.append(np.ascontiguousarray(
            et.reshape(P, HK, 2, NB).transpose(0, 2, 1, 3)  # [P, he, HK, NB]
        ))
    wh_t = tile_rows(Wh_b, HK)
    ws_t = tile_rows(Ws_b, HK)
    for core in range(8):
        b, half = core // 2, core % 2
        in_maps.append(
            {
                "encT": encT_t[b],
                "encf": encf_t[b],
                "decT": tile_rows(
                    np.ascontiguousarray(
                        decoder_state[b, half * DH:(half + 1) * DH].T.astype(bf16)
                    ),
                    HK,
                ),
                "wh": wh_t,
                "ws": ws_t,
                "bsv": bsv,
                "maskl": np.ascontiguousarray(input_pad_mask[b:b + 1]),
            }
        )
    res = run_bass_kernel_spmd(nc, in_maps, core_ids=list(range(8)), trace=trace)

    ctx = np.empty((B, DEC, H), np.float32)
    attn = np.empty((B, DEC, ENC), np.float32)
    for core in range(8):
        b, half = core // 2, core % 2
        ctx[b, half * DH:(half + 1) * DH] = np.asarray(
            res.results[core]["ctx_out"]
        ).astype(np.float32)
        attn[b, half * DH:(half + 1) * DH] = np.asarray(
            res.results[core]["attn_out"]
        ).astype(np.float32)
    if trace:
        kernel.last_result = res
    return ctx, attn


# revision 31
# speedup vs baseline: 1.2064x; 1.2064x over previous
"""Bahdanau additive attention on 8 Trainium2 cores — odd-harmonic kernel.

reference:
    proj_dec = dec @ Ws + bs            [B, DEC, A]
    proj_enc = enc @ Wh                 [B, ENC, A]
    logits[b,d,e] = sum_a v[a] * tanh(proj_dec[b,d,a] + proj_enc[b,e,a])
    attn = renormalized softmax(logits, axis=e) * mask
    ctx = attn @ enc                    [B, DEC, H]
    returns (ctx, attn)

Sharding: 8 cores = (batch b in 0..3) x (decoder half in 0..1); each core does
128 decoder rows against the full encoder of its batch. Fully sync-free: no
cross-core traffic (collectives under this runtime pay a launch-skew barrier
that dwarfs their payload).

Math: tanh(z) ~= sum_{k in {1,3,5,7,9}} b_k sin(k om z) — odd harmonics only
(tanh is a smoothed square wave, so even harmonics contribute ~nothing; L and
b_k are a least-squares fit on [-6.19, 6.19], max fit err 4.9e-3 — half the
error of 8 consecutive harmonics at 5/8 the cost). Angle addition makes the
score one bf16 matmul with contraction A*5*2. Harmonics come from the
stride-2 Chebyshev recurrence with t2 = 2cos(2u):
    s3 = (t2+1) s1,  c3 = (t2-1) c1,  x_k = t2 * x_{k-2} - x_{k-4}
seeded by ACT Sin; t2 and t2+-1 derive from sq = s1^2 via fused tensor_scalar
ops (4x DVE mode). sin/cos chains live interleaved in one tile so each
recurrence step is one elementwise op over both.

Engine split: the e-side chains saturate DVE (the critical resource); the
entire d-side chain pipeline runs on the otherwise-idle GpSimd engine in
parallel. e-side seeds read proj_enc straight out of PSUM (no evacuation
copy). Softmax skips the max-subtraction (|logits| <= sum|v_a|*1.05 ~= 4.3,
f32-exp-safe); Exp emits bf16 + fused row-sum, and 1/sum is folded into the
ctx PSUM evacuation. Inputs are staged host-side as bf16 and pre-transposed
(encT, decT), so no PE transposes feed the projections.
"""

import numpy as np

import concourse.bass as bass
import concourse.mybir as mybir
import concourse.tile as tile
from concourse import bacc
from concourse.bass_utils import run_bass_kernel_spmd
from concourse.masks import make_identity

B, ENC, DEC, H, A = 4, 1024, 256, 1024, 256
P = 128
HK = H // P    # 8 contraction tiles over hidden dim
AT = A // P    # 2 tiles over attention dim
EK = ENC // P  # 8 encoder tiles
DH = 128       # decoder rows per core
NB = 512       # psum bank free-dim (f32)
F32 = mybir.dt.float32
BF16 = mybir.dt.bfloat16
AF = mybir.ActivationFunctionType
ALU = mybir.AluOpType

KS = (1, 3, 5, 7, 9)
NK = len(KS)
OMEGA = float(np.pi / 8.95)

_CACHE = {}


def _fit_coeffs():
    z = np.linspace(-6.19, 6.19, 20001)
    mat = np.sin(np.outer(z, np.array(KS) * OMEGA))
    b = np.linalg.lstsq(mat, np.tanh(z), rcond=None)[0]
    return [float(x) for x in b]


def _build_kernel(mask_ones: bool):
    bco = _fit_coeffs()
    nc = bacc.Bacc("TRN2", target_bir_lowering=False, debug=False)
    # all big inputs are host-tiled to [P, ...contiguous] so each DMA is 128
    # large descriptors instead of ~1024 small ones
    encT = nc.dram_tensor("encT", [P, 2, HK, NB], BF16, kind="ExternalInput").ap()
    encf = nc.dram_tensor("encf", [P, EK, H], BF16, kind="ExternalInput").ap()
    decT = nc.dram_tensor("decT", [P, HK, DH], BF16, kind="ExternalInput").ap()
    wh = nc.dram_tensor("wh", [P, HK, A], BF16, kind="ExternalInput").ap()
    ws = nc.dram_tensor("ws", [P, HK, A], BF16, kind="ExternalInput").ap()
    bsv = nc.dram_tensor("bsv", [P, 2 * AT], F32, kind="ExternalInput").ap()
    maskl = nc.dram_tensor("maskl", [1, ENC], F32, kind="ExternalInput").ap()
    ctx_out = nc.dram_tensor("ctx_out", [DH, H], BF16, kind="ExternalOutput").ap()
    attn_out = nc.dram_tensor("attn_out", [DH, ENC], BF16, kind="ExternalOutput").ap()

    def bcast(t, n):
        return bass.AP(tensor=t.tensor, offset=t.offset, ap=[[0, P], [1, n]])

    with tile.TileContext(nc) as tc:
        with (
            tc.tile_pool(name="big", bufs=1) as big,
            tc.tile_pool(name="small", bufs=1) as small,
            tc.tile_pool(name="ech", bufs=5) as ech,
            tc.tile_pool(name="ps_mm", bufs=3, space="PSUM") as ps_mm,
            tc.tile_pool(name="ps_lg", bufs=1, space="PSUM") as ps_lg,
            tc.tile_pool(name="ps_w", bufs=1, space="PSUM") as ps_w,
            tc.tile_pool(name="ps_tr", bufs=1, space="PSUM") as ps_tr,
            tc.tile_pool(name="ps_cx", bufs=1, space="PSUM") as ps_cx,
        ):
            # ---- loads. Small d-side tensors first (scalar queue) so pd can
            # start immediately; encT halves next (sync); encf is dispatched
            # much later in program order — ctx needs it only near the end,
            # and an early dispatch would starve everything else on the bus.
            decT_sb = big.tile([P, HK, DH], BF16)
            nc.gpsimd.dma_start(out=decT_sb, in_=decT)
            ws_sb = big.tile([P, HK, A], BF16)
            nc.gpsimd.dma_start(out=ws_sb, in_=ws)
            bsv_sb = small.tile([P, 2 * AT], F32)
            nc.scalar.dma_start(out=bsv_sb, in_=bsv)
            wh_sb = big.tile([P, HK, A], BF16)
            nc.gpsimd.dma_start(out=wh_sb, in_=wh)
            bs_sb = bsv_sb[:, 0:AT]
            v_sb = bsv_sb[:, AT:2 * AT]
            encT_sb = big.tile([P, 2, HK, NB], BF16)
            nc.sync.dma_start(out=encT_sb[:, 0], in_=encT[:, 0])
            nc.sync.dma_start(out=encT_sb[:, 1], in_=encT[:, 1])
            encf_sb = big.tile([P, EK, H], BF16)
            nc.sync.dma_start(out=encf_sb, in_=encf)
            if not mask_ones:
                mask_sb = big.tile([P, ENC], F32)
                nc.sync.dma_start(out=mask_sb, in_=bcast(maskl, ENC))

            consts = small.tile([P, 2], F32)
            nc.vector.memset(consts[:, 0:1], OMEGA)
            nc.vector.memset(consts[:, 1:2], float(np.pi / 2))
            om_ap = consts[:, 0:1]
            halfpi_ap = consts[:, 1:2]

            fd = big.tile([P, AT, NK, 2, DH], BF16)

            def pe_warm(n):
                for _ in range(n):
                    pw = ps_w.tile([P, NB], F32, tag="warm")
                    nc.tensor.matmul(
                        pw[:, :2 * DH], fd[:, 0, 0, 0], fd[:, 0, 0], start=True,
                        stop=True, skip_group_check=True,
                    )

            pe_warm(4)

            def chain_steps(eng, sc1, n, pool, tag, slices, mm_hook=None):
                """Odd-harmonic sin/cos arrays from seeded sc1 (see module doc).

                Returns {k: [P, AT, 2, n] bf16}; recurrence math runs on `eng`,
                emitted independently per column slice so each slice's chain
                (and its logits matmuls) can start as soon as that slice's
                seeds exist.
                """
                sq = big.tile([P, AT, n], BF16, tag=tag + "sq")
                t2 = big.tile([P, AT, 2, n], BF16, tag=tag + "t2")
                t2pm = big.tile([P, AT, 2, n], BF16, tag=tag + "t2pm")
                sc3 = pool.tile([P, AT, 2, n], BF16, tag=tag)
                sc = {1: sc1, 3: sc3}
                for k in (5, 7, 9):
                    sc[k] = pool.tile([P, AT, 2, n], BF16, tag=tag, name=f"{tag}sc{k}")
                # steps interleave the (independent) column slices so the
                # in-order engine fills one slice's recurrence latency with
                # the other slice's work. DVE tensor_scalar runs 1x on HW, so
                # the t2 builders go to ACT as Copy(scale,bias) for the big
                # e-side; gpsimd keeps its own (immediate scalars are fine
                # there).
                for si, sl in enumerate(slices):
                    eng.tensor_mul(sq[:, :, sl], sc1[:, :, 0, sl], sc1[:, :, 0, sl])
                    if mm_hook:
                        mm_hook(0, sc1, si)
                for si, sl in enumerate(slices):
                    for ph in range(2):
                        eng.tensor_scalar(
                            out=t2pm[:, :, ph, sl], in0=sq[:, :, sl], scalar1=-4.0,
                            scalar2=(3.0 if ph == 0 else 1.0), op0=ALU.mult, op1=ALU.add,
                        )
                    for ph in range(2):
                        eng.tensor_scalar(
                            out=t2[:, :, ph, sl], in0=sq[:, :, sl], scalar1=-4.0,
                            scalar2=2.0, op0=ALU.mult, op1=ALU.add,
                        )
                for si, sl in enumerate(slices):
                    eng.tensor_mul(sc3[:, :, :, sl], t2pm[:, :, :, sl], sc1[:, :, :, sl])
                    if mm_hook:
                        mm_hook(1, sc3, si)
                for ki, k in enumerate((5, 7, 9)):
                    t = sc[k]
                    for si, sl in enumerate(slices):
                        eng.tensor_mul(t[:, :, :, sl], t2[:, :, :, sl], sc[k - 2][:, :, :, sl])
                    for si, sl in enumerate(slices):
                        eng.tensor_sub(t[:, :, :, sl], t[:, :, :, sl], sc[k - 4][:, :, :, sl])
                        if mm_hook:
                            mm_hook(2 + ki, t, si)
                return sc

            # ---- d-side: proj_dec^T [a,(at,d)]; chains on GpSimd so DVE
            # stays free for the (4x bigger) e-side chains.
            pd_sb = big.tile([P, AT, DH], F32)
            for at in range(AT):
                pp = ps_mm.tile([P, NB], F32, tag="mm")
                for hk in range(HK):
                    nc.tensor.matmul(
                        pp[:, :DH],
                        ws_sb[:, hk, at * P:(at + 1) * P],
                        decT_sb[:, hk, :],
                        start=(hk == 0),
                        stop=(hk == HK - 1),
                    )
                nc.vector.tensor_scalar_add(pd_sb[:, at], pp[:, :DH], bs_sb[:, at:at + 1])

            pe_warm(6)
            dsc1 = ech.tile([P, AT, 2, DH], BF16, tag="d")
            nc.scalar.activation(out=dsc1[:, :, 0], in_=pd_sb, func=AF.Sin, scale=om_ap)
            nc.scalar.activation(out=dsc1[:, :, 1], in_=pd_sb, func=AF.Sin, scale=om_ap, bias=halfpi_ap)
            vb = small.tile([P, AT, NK], F32)
            for ki in range(NK):
                for at in range(AT):
                    nc.vector.tensor_scalar_mul(
                        vb[:, at, ki:ki + 1], v_sb[:, at:at + 1], bco[ki]
                    )
            dsc = chain_steps(nc.gpsimd, dsc1, DH, ech, "d", [slice(0, DH)])
            for ki in range(NK):
                for at in range(AT):
                    nc.vector.tensor_scalar_mul(
                        fd[:, at, ki], dsc[KS[ki]][:, at], vb[:, at, ki:ki + 1]
                    )

            # ---- e-side: proj_enc^T per PSUM quadrant, seeds read PSUM ----
            esc1 = ech.tile([P, AT, 2, ENC], BF16, tag="e")
            for he in range(2):
                for at in range(AT):
                    pp = ps_mm.tile([P, NB], F32, tag="mm")
                    for hk in range(HK):
                        nc.tensor.matmul(
                            pp,
                            wh_sb[:, hk, at * P:(at + 1) * P],
                            encT_sb[:, he, hk, :],
                            start=(hk == 0),
                            stop=(hk == HK - 1),
                        )
                    sl = slice(he * NB, (he + 1) * NB)
                    nc.scalar.activation(out=esc1[:, at, 0, sl], in_=pp, func=AF.Sin, scale=om_ap)
                    nc.scalar.activation(out=esc1[:, at, 1, sl], in_=pp, func=AF.Sin, scale=om_ap, bias=halfpi_ap)

            # ---- e-side chains with logits matmuls interleaved ----
            lg_psum = ps_lg.tile([P, 2, NB], F32)

            def logits_mm(ki, esc_k, he):
                for ph in range(2):
                    for at in range(AT):
                        nc.tensor.matmul(
                            lg_psum[:, he],
                            fd[:, at, ki, ph],
                            esc_k[:, at, 1 - ph, he * NB:(he + 1) * NB],
                            start=(ki == 0 and ph == 0 and at == 0),
                            stop=(ki == NK - 1 and ph == 1 and at == AT - 1),
                            skip_group_check=True,
                        )

            chain_steps(
                nc.vector, esc1, ENC, ech, "e",
                [slice(0, NB), slice(NB, ENC)], mm_hook=logits_mm,
            )

            # ---- softmax, without max-subtraction: |logits| <= sum|v_a|*1.05
            # (~4.3 here), so exp() is f32-safe unshifted. Exp writes bf16
            # directly and emits the row-sum in the same pass; 1/sum is folded
            # into the ctx PSUM evacuation and a parallel attn-output scale.
            pe_warm(8)
            expt_bf = big.tile([P, ENC], BF16)
            rowsum = small.tile([P, 1], F32)
            if mask_ones:
                nc.scalar.activation(
                    out=expt_bf, in_=lg_psum.rearrange("p h e -> p (h e)"),
                    func=AF.Exp, accum_out=rowsum,
                )
            else:
                nc.scalar.activation(
                    out=expt_bf, in_=lg_psum.rearrange("p h e -> p (h e)"), func=AF.Exp
                )
                nc.vector.tensor_mul(expt_bf, expt_bf, mask_sb)
                nc.vector.tensor_reduce(
                    out=rowsum, in_=expt_bf, axis=mybir.AxisListType.X, op=ALU.add
                )
            rinv = small.tile([P, 1], F32)
            nc.vector.reciprocal(rinv, rowsum)
            attn_bf = big.tile([P, ENC], BF16)
            nc.scalar.mul(attn_bf, expt_bf, rinv)
            nc.sync.dma_start(out=attn_out, in_=attn_bf)

            # ---- ctx = attn @ enc (unnormalized; rinv applied at evac) ----
            ident_f = small.tile([P, P], F32)
            make_identity(nc, ident_f)
            ident = small.tile([P, P], BF16)
            nc.vector.tensor_copy(ident, ident_f)
            attnT = big.tile([P, EK, P], BF16)
            for g in range(2):
                pt = ps_tr.tile([P, 4, P], BF16)
                for j in range(4):
                    ek = g * 4 + j
                    nc.tensor.transpose(pt[:, j], expt_bf[:, ek * P:(ek + 1) * P], ident)
                for j in range(4):
                    nc.vector.tensor_copy(attnT[:, g * 4 + j], pt[:, j])
            ctx_sb = big.tile([P, H], BF16)
            for nh in range(2):
                pc = ps_cx.tile([P, NB], F32, tag="cx")
                for ek in range(EK):
                    nc.tensor.matmul(
                        pc,
                        attnT[:, ek],
                        encf_sb[:, ek, nh * NB:(nh + 1) * NB],
                        start=(ek == 0),
                        stop=(ek == EK - 1),
                    )
                nc.scalar.mul(ctx_sb[:, nh * NB:(nh + 1) * NB], pc, rinv)
                nc.sync.dma_start(
                    out=bass.AP(
                        tensor=ctx_out.tensor, offset=ctx_out.offset + nh * NB,
                        ap=[[H, P], [1, NB]],
                    ),
                    in_=ctx_sb[:, nh * NB:(nh + 1) * NB],
                )

    nc.compile()
    return nc


def kernel(encoded_seq, decoder_state, input_pad_mask, Wh, Ws, bs, v, trace=False):
    import ml_dtypes

    bf16 = ml_dtypes.bfloat16
    encoded_seq = np.asarray(encoded_seq, dtype=np.float32)
    decoder_state = np.asarray(decoder_state, dtype=np.float32)
    input_pad_mask = np.asarray(input_pad_mask, dtype=np.float32)
    Wh_b = np.ascontiguousarray(np.asarray(Wh, np.float32).astype(bf16))
    Ws_b = np.ascontiguousarray(np.asarray(Ws, np.float32).astype(bf16))
    bs2 = np.asarray(bs, dtype=np.float32).reshape(AT, P)
    v2 = np.asarray(v, dtype=np.float32).reshape(AT, P)
    # host-tiled [P, (bs_at0, bs_at1, v_at0, v_at1)] — plain contiguous load
    bsv = np.ascontiguousarray(np.concatenate([bs2.T, v2.T], axis=1))

    mask_ones = bool(np.all(input_pad_mask == 1.0))
    key = ("nc", mask_ones)
    if key not in _CACHE:
        _CACHE[key] = _build_kernel(mask_ones)
    nc = _CACHE[key]

    def tile_rows(x, k):
        # [k*P, n] -> [P, k, n] per-partition-contiguous
        n = x.shape[1]
        return np.ascontiguousarray(x.reshape(k, P, n).transpose(1, 0, 2))

    in_maps = []
    enc_bf = [encoded_seq[b].astype(bf16) for b in range(B)]
    encf_t = [tile_rows(e, EK) for e in enc_bf]
    encT_t = []
    for e in enc_bf:
        et = tile_rows(np.ascontiguousarray(e.T), HK)       # [P, HK, ENC]
        encT_t.append(np.ascontiguousarray(
            et.reshape(P, HK, 2, NB).transpose(0, 2, 1, 3)  # [P, he, HK, NB]
        ))
    wh_t = tile_rows(Wh_b, HK)
    ws_t = tile_rows(Ws_b, HK)
    for core in range(8):
        b, half = core // 2, core % 2
        in_maps.append(
            {
                "encT": encT_t[b],
                "encf": encf_t[b],
                "decT": tile_rows(
                    np.ascontiguousarray(
                        decoder_state[b, half * DH:(half + 1) * DH].T.astype(bf16)
                    ),
                    HK,
                ),
                "wh": wh_t,
                "ws": ws_t,
                "bsv": bsv,
                "maskl": np.ascontiguousarray(input_pad_mask[b:b + 1]),
            }
        )
    res = run_bass_kernel_spmd(nc, in_maps, core_ids=list(range(8)), trace=trace)

    ctx = np.empty((B, DEC, H), np.float32)
    attn = np.empty((B, DEC, ENC), np.float32)
    for core in range(8):
        b, half = core // 2, core % 2
        ctx[b, half * DH:(half + 1) * DH] = np.asarray(
            res.results[core]["ctx_out"]
        ).astype(np.float32)
        attn[b, half * DH:(half + 1) * DH] = np.asarray(
            res.results[core]["attn_out"]
        ).astype(np.float32)
    if trace:
        kernel.last_result = res
    return ctx, attn
